# revision 5
# baseline (speedup 1.0000x reference)
"""VQ codebook kernel for Trainium2 (8 NeuronCores, data-parallel over batch).

Computes, for z [32,256,32,32] and codebook weight [1024,256]:
  - z_q_out [32,256,32,32]: nearest-codebook-entry vectors (straight-through
    output equals the quantized values in forward)
  - loss scalar: (1+0.25) * mean((z_nhwc - z_q)^2)
  - min_indices [32768] int32

Distance uses d = ||z||^2 + ||e||^2 - 2 z.e with the same fp32 rounding
structure as the jax reference (the ||z||^2 bias quantizes d at ~1.5e-5,
which is what makes argmin tie-breaks reproducible), computed as
  td = (-w2_bcast + -||z||^2_col) - psum(z @ (-2 w^T))  = -d
then per-row max8/max_index gives argmin with first-index tie-break,
matching jnp.argmin. z_q rows are fetched with an indirect DMA gather and
transposed back to NCHW via the tensor engine.

Sharding: z split on batch into 8 shards of [4,256,32,32]; weight replicated.
Per-core partial loss sums are combined on the host (scalar unshard).
"""

import numpy as np

B_LOC = 4          # batches per core
C = 256            # embed dim
HW = 1024          # 32*32
K = 1024           # codebook size
N_CORES = 8
BETA = 0.25

_BUILT = {}


def _split_excess_waits(nc, max_waits=1):
    """walrus in this container rejects instructions carrying more than a
    couple of semaphore waits (the 8-byte Events encoding). Hoist excess
    waits onto same-engine NOPs immediately before the instruction —
    engine queues are FIFO, so blocking semantics are identical."""
    import concourse.mybir as mybir

    for f in nc.m.functions:
        for bb in f.blocks:
            new = []
            for inst in bb.instructions:
                si = inst.sync_info
                if si and si.on_wait and len(si.on_wait) > max_waits:
                    waits = list(si.on_wait)
                    for w in waits[:-max_waits]:
                        nop = mybir.InstNoOp(
                            name=nc.get_next_instruction_name(), ins=[], outs=[])
                        nop.engine = inst.engine
                        nop.sync_info = mybir.SyncInfo(on_wait=[w], on_update=[])
                        new.append(nop)
                    inst.sync_info = mybir.SyncInfo(
                        on_wait=waits[-max_waits:],
                        on_update=list(si.on_update or []))
                new.append(inst)
            bb.instructions[:] = new


def _build():
    from contextlib import ExitStack
    import concourse.bass as bass
    import concourse.tile as tile
    from concourse import mybir
    from concourse.masks import make_identity

    f32 = mybir.dt.float32
    u32 = mybir.dt.uint32
    i32 = mybir.dt.int32
    Op = mybir.AluOpType

    nc = bass.Bass("TRN2", target_bir_lowering=False, debug=False)
    z_ap = nc.dram_tensor("z", [B_LOC, C, HW], f32, kind="ExternalInput").ap()
    w_ap = nc.dram_tensor("w", [K, C], f32, kind="ExternalInput").ap()
    zq_ap = nc.dram_tensor("zq", [B_LOC, C, HW], f32, kind="ExternalOutput").ap()
    idx_ap = nc.dram_tensor("idx", [B_LOC, HW], i32, kind="ExternalOutput").ap()
    lacc_ap = nc.dram_tensor("lacc", [128, 1], f32, kind="ExternalOutput").ap()

    with tile.TileContext(nc) as tc, ExitStack() as ctx:
        const = ctx.enter_context(tc.tile_pool(name="const", bufs=1))
        zpool = ctx.enter_context(tc.tile_pool(name="zb", bufs=2))
        sqpool = ctx.enter_context(tc.tile_pool(name="zsq", bufs=2))
        rowpool = ctx.enter_context(tc.tile_pool(name="rows", bufs=2))
        apool = ctx.enter_context(tc.tile_pool(name="acol", bufs=2))
        tdpool = ctx.enter_context(tc.tile_pool(name="td", bufs=3))
        mpool = ctx.enter_context(tc.tile_pool(name="maxv", bufs=3))
        ipool = ctx.enter_context(tc.tile_pool(name="idx8", bufs=3))
        gpool = ctx.enter_context(tc.tile_pool(name="zqg", bufs=4))
        qpool = ctx.enter_context(tc.tile_pool(name="zqt", bufs=2))
        dpool = ctx.enter_context(tc.tile_pool(name="diff", bufs=2))
        opool = ctx.enter_context(tc.tile_pool(name="zqo", bufs=2))
        pd = ctx.enter_context(tc.tile_pool(name="pd", bufs=2, space="PSUM"))
        pt = ctx.enter_context(tc.tile_pool(name="pt", bufs=2, space="PSUM"))
        pa = ctx.enter_context(tc.tile_pool(name="pa", bufs=2, space="PSUM"))

        identity = const.tile([128, 128], f32)
        make_identity(nc, identity[:])
        onesc = const.tile([128, 1], f32)
        nc.vector.memset(onesc[:], 1.0)
        onesr = const.tile([1, 128], f32)
        nc.vector.memset(onesr[:], 1.0)
        lacc = const.tile([128, 1], f32)
        nc.vector.memset(lacc[:], 0.0)

        # --- one-time: load w, build wTm2 = -2 * w^T  (2 c-chunks x 1024 k) ---
        wnat = const.tile([128, 8 * C], f32)       # k-chunk kc at cols [kc*256, +256)
        for kc in range(8):
            nc.sync.dma_start(wnat[:, kc * C:(kc + 1) * C], w_ap[kc * 128:(kc + 1) * 128, :])
        wTm2 = const.tile([128, 2 * K], f32)       # c-chunk cc at cols [cc*1024, +1024)
        for kc in range(8):
            for cc in range(2):
                blk = pt.tile([128, 128], f32, tag="pt")
                nc.tensor.transpose(blk[:], wnat[:, kc * C + cc * 128: kc * C + (cc + 1) * 128], identity[:])
                nc.scalar.mul(wTm2[:, cc * K + kc * 128: cc * K + (kc + 1) * 128], blk[:], -2.0)
        # wTsq = (wTm2)^2 = 4 w^2
        wTsq = const.tile([128, 2 * K], f32)
        nc.scalar.square(wTsq[:, :K], wTm2[:, :K])
        nc.scalar.square(wTsq[:, K:], wTm2[:, K:])
        # w2 row = sum_c w^2 (ones-matmul over partitions), then negate (scale -0.25)
        prow = pd.tile([1, K], f32, tag="pd")
        for nh in range(2):
            for cc in range(2):
                nc.tensor.matmul(prow[0:1, nh * 512:(nh + 1) * 512],
                                 lhsT=onesc[:, 0:1],
                                 rhs=wTsq[:, cc * K + nh * 512: cc * K + (nh + 1) * 512],
                                 start=(cc == 0), stop=(cc == 1))
        negw2row = const.tile([1, K], f32)
        nc.scalar.mul(negw2row[:], prow[:], -0.25)
        # negW2B: broadcast -w2 row to 128 partitions via K=1 matmul
        pb = pd.tile([128, K], f32, tag="pd")
        for nh in range(2):
            nc.tensor.matmul(pb[:, nh * 512:(nh + 1) * 512],
                             lhsT=onesr[0:1, :],
                             rhs=negw2row[0:1, nh * 512:(nh + 1) * 512],
                             start=True, stop=True)
        negW2B = const.tile([128, K], f32)
        nc.scalar.copy(negW2B[:], pb[:])

        for b in range(B_LOC):
            zb = zpool.tile([128, 2 * HW], f32)    # c-chunk cc at cols [cc*1024, +1024)
            nc.sync.dma_start(zb[:, :HW], z_ap[b, 0:128, :])
            nc.sync.dma_start(zb[:, HW:], z_ap[b, 128:256, :])

            # A row: ||z_n||^2 for the 1024 vectors of this batch
            zsq = sqpool.tile([128, 2 * HW], f32)
            nc.scalar.square(zsq[:, :HW], zb[:, :HW])
            nc.scalar.square(zsq[:, HW:], zb[:, HW:])
            prowA = pd.tile([1, HW], f32, tag="pd")
            for nh in range(2):
                for cc in range(2):
                    nc.tensor.matmul(prowA[0:1, nh * 512:(nh + 1) * 512],
                                     lhsT=onesc[:, 0:1],
                                     rhs=zsq[:, cc * HW + nh * 512: cc * HW + (nh + 1) * 512],
                                     start=(cc == 0), stop=(cc == 1))
            negArow = rowpool.tile([1, HW], f32)
            nc.scalar.mul(negArow[:], prowA[:], -1.0)
            # transpose row -> per-m_tile columns [128, 8] via K=1 matmuls
            pacol = pa.tile([128, 8], f32, tag="pa")
            for j in range(8):
                nc.tensor.matmul(pacol[:, j:j + 1],
                                 lhsT=negArow[0:1, j * 128:(j + 1) * 128],
                                 rhs=onesr[0:1, 0:1],
                                 start=True, stop=True)
            negAcol = apool.tile([128, 8], f32)
            nc.scalar.copy(negAcol[:], pacol[:])

            zqTb = qpool.tile([128, 2 * HW], f32)  # c-chunk cc at cols [cc*1024, +1024)
            for j in range(8):
                pdd = pd.tile([128, K], f32, tag="pd")
                for nh in range(2):
                    nc.tensor.matmul(pdd[:, nh * 512:(nh + 1) * 512],
                                     lhsT=zb[:, 0 * HW + j * 128: 0 * HW + (j + 1) * 128],
                                     rhs=wTm2[:, 0 * K + nh * 512: 0 * K + (nh + 1) * 512],
                                     start=True, stop=False)
                    nc.tensor.matmul(pdd[:, nh * 512:(nh + 1) * 512],
                                     lhsT=zb[:, 1 * HW + j * 128: 1 * HW + (j + 1) * 128],
                                     rhs=wTm2[:, 1 * K + nh * 512: 1 * K + (nh + 1) * 512],
                                     start=False, stop=True)
                # td = (-w2 + -A) - (-2 z.e) = -d   (rounding matches reference)
                td = tdpool.tile([128, K], f32)
                nc.vector.scalar_tensor_tensor(out=td[:], in0=negW2B[:],
                                               scalar=negAcol[:, j:j + 1],
                                               in1=pdd[:],
                                               op0=Op.add, op1=Op.subtract)
                maxv = mpool.tile([128, 8], f32)
                nc.vector.max(maxv[:], td[:])
                idx8 = ipool.tile([128, 8], u32)
                nc.vector.max_index(idx8[:], maxv[:], td[:])
                # loss accumulator: sum of (-d_min)
                nc.vector.tensor_add(lacc[:], lacc[:], maxv[:, 0:1])
                nc.sync.dma_start(idx_ap[b, j * 128:(j + 1) * 128], idx8[:, 0:1].bitcast(i32))
                # gather z_q rows, transpose to [c, n], stage for output
                zqg = gpool.tile([128, C], f32)
                nc.gpsimd.indirect_dma_start(
                    out=zqg[:], out_offset=None, in_=w_ap[:, :],
                    in_offset=bass.IndirectOffsetOnAxis(ap=idx8[:, 0:1], axis=0))
                for cc in range(2):
                    ptt = pt.tile([128, 128], f32, tag="pt")
                    nc.tensor.transpose(ptt[:], zqg[:, cc * 128:(cc + 1) * 128], identity[:])
                    nc.scalar.copy(zqTb[:, cc * HW + j * 128: cc * HW + (j + 1) * 128], ptt[:])
            # straight-through output: z + (z_q - z) with reference fp32 rounding
            diff = dpool.tile([128, 2 * HW], f32)
            nc.vector.tensor_sub(diff[:], zqTb[:], zb[:])
            zqo = opool.tile([128, 2 * HW], f32)
            nc.vector.tensor_add(zqo[:], zb[:], diff[:])
            for cc in range(2):
                nc.sync.dma_start(zq_ap[b, cc * 128:(cc + 1) * 128, :], zqo[:, cc * HW:(cc + 1) * HW])

        nc.sync.dma_start(lacc_ap[:], lacc[:])

    _split_excess_waits(nc)
    return nc


def _get_nc():
    if "nc" not in _BUILT:
        _BUILT["nc"] = _build()
    return _BUILT["nc"]


def kernel(z, weight):
    from concourse.bass_utils import run_bass_kernel_spmd

    z = np.ascontiguousarray(np.asarray(z), dtype=np.float32)
    weight = np.ascontiguousarray(np.asarray(weight), dtype=np.float32)
    B = z.shape[0]
    assert z.shape == (B, C, 32, 32) and weight.shape == (K, C)
    zr = z.reshape(B, C, HW)

    nc = _get_nc()
    in_maps = [{"z": zr[c * B_LOC:(c + 1) * B_LOC], "w": weight} for c in range(N_CORES)]
    res = run_bass_kernel_spmd(nc, in_maps, core_ids=list(range(N_CORES)))

    zq = np.concatenate([res.results[c]["zq"] for c in range(N_CORES)], axis=0)
    zq = zq.reshape(B, C, 32, 32)
    idx = np.concatenate([res.results[c]["idx"].reshape(-1) for c in range(N_CORES)]).astype(np.int32)
    s = -sum(res.results[c]["lacc"].astype(np.float64).sum() for c in range(N_CORES))
    loss = np.float32(1.25 * s / (B * C * HW))
    return zq, loss, idx


# revision 8
# speedup vs baseline: 1.1386x; 1.1386x over previous
"""VQ codebook kernel for Trainium2 (8 NeuronCores, data-parallel over batch).

Computes, for z [32,256,32,32] and codebook weight [1024,256]:
  - z_q_out [32,256,32,32]: nearest-codebook-entry vectors (straight-through
    output equals the quantized values in forward)
  - loss scalar: (1+0.25) * mean((z_nhwc - z_q)^2)
  - min_indices [32768] int32

Distance uses d = ||z||^2 + ||e||^2 - 2 z.e with the same fp32 rounding
structure as the jax reference (the ||z||^2 bias quantizes d at ~1.5e-5,
which is what makes argmin tie-breaks reproducible), computed as
  td = (-w2_bcast + -||z||^2_col) - psum(z @ (-2 w^T))  = -d
then per-row max8/max_index gives argmin with first-index tie-break,
matching jnp.argmin. z_q rows are fetched with an indirect DMA gather and
transposed back to NCHW via the tensor engine.

Sharding: z split on batch into 8 shards of [4,256,32,32]; weight replicated.
Per-core partial loss sums are combined on the host (scalar unshard).
"""

import numpy as np

B_LOC = 4          # batches per core
C = 256            # embed dim
HW = 1024          # 32*32
K = 1024           # codebook size
N_CORES = 8
BETA = 0.25

_BUILT = {}


def _split_excess_waits(nc, max_waits=1):
    """walrus in this container rejects instructions carrying more than a
    couple of semaphore waits (the 8-byte Events encoding). Hoist excess
    waits onto same-engine NOPs immediately before the instruction —
    engine queues are FIFO, so blocking semantics are identical."""
    import concourse.mybir as mybir

    for f in nc.m.functions:
        for bb in f.blocks:
            new = []
            for inst in bb.instructions:
                si = inst.sync_info
                if si and si.on_wait and len(si.on_wait) > max_waits:
                    waits = list(si.on_wait)
                    for w in waits[:-max_waits]:
                        nop = mybir.InstNoOp(
                            name=nc.get_next_instruction_name(), ins=[], outs=[])
                        nop.engine = inst.engine
                        nop.sync_info = mybir.SyncInfo(on_wait=[w], on_update=[])
                        new.append(nop)
                    inst.sync_info = mybir.SyncInfo(
                        on_wait=waits[-max_waits:],
                        on_update=list(si.on_update or []))
                new.append(inst)
            bb.instructions[:] = new


def _build():
    from contextlib import ExitStack
    import concourse.bass as bass
    import concourse.tile as tile
    from concourse import mybir
    from concourse.masks import make_identity

    f32 = mybir.dt.float32
    u32 = mybir.dt.uint32
    i32 = mybir.dt.int32
    Op = mybir.AluOpType
    Act = mybir.ActivationFunctionType

    nc = bass.Bass("TRN2", target_bir_lowering=False, debug=False)
    z_ap = nc.dram_tensor("z", [B_LOC, C, HW], f32, kind="ExternalInput").ap()
    w_ap = nc.dram_tensor("w", [K, C], f32, kind="ExternalInput").ap()
    zq_ap = nc.dram_tensor("zq", [B_LOC, C, HW], f32, kind="ExternalOutput").ap()
    idx_ap = nc.dram_tensor("idx", [B_LOC, HW], i32, kind="ExternalOutput").ap()
    lacc_ap = nc.dram_tensor("lacc", [B_LOC * 8, 128], f32, kind="ExternalOutput").ap()

    with tile.TileContext(nc) as tc, ExitStack() as ctx:
        const = ctx.enter_context(tc.tile_pool(name="const", bufs=1))
        zpool = ctx.enter_context(tc.tile_pool(name="zb", bufs=2))
        sqpool = ctx.enter_context(tc.tile_pool(name="zsq", bufs=2))
        rowpool = ctx.enter_context(tc.tile_pool(name="rows", bufs=2))
        apool = ctx.enter_context(tc.tile_pool(name="acol", bufs=2))
        upool = ctx.enter_context(tc.tile_pool(name="ubias", bufs=3))
        tdpool = ctx.enter_context(tc.tile_pool(name="td", bufs=3))
        mpool = ctx.enter_context(tc.tile_pool(name="maxv", bufs=3))
        ipool = ctx.enter_context(tc.tile_pool(name="idx8", bufs=3))
        ibpool = ctx.enter_context(tc.tile_pool(name="idxb", bufs=2))
        gpool = ctx.enter_context(tc.tile_pool(name="zqg", bufs=2))
        qpool = ctx.enter_context(tc.tile_pool(name="zqt", bufs=2))
        dpool = ctx.enter_context(tc.tile_pool(name="diff", bufs=2))
        opool = ctx.enter_context(tc.tile_pool(name="zqo", bufs=2))
        pd = ctx.enter_context(tc.tile_pool(name="pd", bufs=2, space="PSUM"))
        pt = ctx.enter_context(tc.tile_pool(name="pt", bufs=2, space="PSUM"))
        pa = ctx.enter_context(tc.tile_pool(name="pa", bufs=2, space="PSUM"))

        identity = const.tile([128, 128], f32)
        make_identity(nc, identity[:])
        onesc = const.tile([128, 1], f32)
        nc.vector.memset(onesc[:], 1.0)
        onesr = const.tile([1, 128], f32)
        nc.vector.memset(onesr[:], 1.0)

        # --- one-time: load w, build wTp2 = +2 * w^T  (2 c-chunks x 1024 k) ---
        wnat = const.tile([128, 8 * C], f32)       # k-chunk kc at cols [kc*256, +256)
        for kc in range(8):
            nc.sync.dma_start(wnat[:, kc * C:(kc + 1) * C], w_ap[kc * 128:(kc + 1) * 128, :])
        wTp2 = const.tile([128, 2 * K], f32)       # c-chunk cc at cols [cc*1024, +1024)
        for kc in range(8):
            for cc in range(2):
                blk = pt.tile([128, 128], f32, tag="pt")
                nc.tensor.transpose(blk[:], wnat[:, kc * C + cc * 128: kc * C + (cc + 1) * 128], identity[:])
                nc.scalar.mul(wTp2[:, cc * K + kc * 128: cc * K + (kc + 1) * 128], blk[:], 2.0)
        # wTsq = (wTp2)^2 = 4 w^2
        wTsq = const.tile([128, 2 * K], f32)
        nc.scalar.square(wTsq[:, :K], wTp2[:, :K])
        nc.scalar.square(wTsq[:, K:], wTp2[:, K:])
        # w2 row = sum_c w^2 (ones-matmul over partitions; scale 0.25 undoes the 4x)
        prow = pd.tile([1, K], f32, tag="pd")
        for nh in range(2):
            for cc in range(2):
                nc.tensor.matmul(prow[0:1, nh * 512:(nh + 1) * 512],
                                 lhsT=onesc[:, 0:1],
                                 rhs=wTsq[:, cc * K + nh * 512: cc * K + (nh + 1) * 512],
                                 start=(cc == 0), stop=(cc == 1))
        w2row = const.tile([1, K], f32)
        nc.scalar.mul(w2row[:], prow[:], 0.25)
        # W2B: broadcast w2 row to 128 partitions via K=1 matmul
        pb = pd.tile([128, K], f32, tag="pd")
        for nh in range(2):
            nc.tensor.matmul(pb[:, nh * 512:(nh + 1) * 512],
                             lhsT=onesr[0:1, :],
                             rhs=w2row[0:1, nh * 512:(nh + 1) * 512],
                             start=True, stop=True)
        W2B = const.tile([128, K], f32)
        nc.scalar.copy(W2B[:], pb[:])

        for b in range(B_LOC):
            zb = zpool.tile([128, 2 * HW], f32)    # c-chunk cc at cols [cc*1024, +1024)
            nc.sync.dma_start(zb[:, :HW], z_ap[b, 0:128, :])
            nc.sync.dma_start(zb[:, HW:], z_ap[b, 128:256, :])

            # A row: ||z_n||^2 for the 1024 vectors of this batch
            zsq = sqpool.tile([128, 2 * HW], f32)
            nc.scalar.square(zsq[:, :HW], zb[:, :HW])
            nc.scalar.square(zsq[:, HW:], zb[:, HW:])
            prowA = pd.tile([1, HW], f32, tag="pd")
            for nh in range(2):
                for cc in range(2):
                    nc.tensor.matmul(prowA[0:1, nh * 512:(nh + 1) * 512],
                                     lhsT=onesc[:, 0:1],
                                     rhs=zsq[:, cc * HW + nh * 512: cc * HW + (nh + 1) * 512],
                                     start=(cc == 0), stop=(cc == 1))
            Arow = rowpool.tile([1, HW], f32)
            nc.scalar.copy(Arow[:], prowA[:])
            # transpose row -> per-m_tile columns [128, 8] via K=1 matmuls
            pacol = pa.tile([128, 8], f32, tag="pa")
            for j in range(8):
                nc.tensor.matmul(pacol[:, j:j + 1],
                                 lhsT=Arow[0:1, j * 128:(j + 1) * 128],
                                 rhs=onesr[0:1, 0:1],
                                 start=True, stop=True)
            Acol = apool.tile([128, 8], f32)
            nc.scalar.copy(Acol[:], pacol[:])

            zqg = gpool.tile([128, 8, C], f32)
            for j in range(8):
                pdd = pd.tile([128, K], f32, tag="pd")
                for nh in range(2):
                    nc.tensor.matmul(pdd[:, nh * 512:(nh + 1) * 512],
                                     lhsT=zb[:, 0 * HW + j * 128: 0 * HW + (j + 1) * 128],
                                     rhs=wTp2[:, 0 * K + nh * 512: 0 * K + (nh + 1) * 512],
                                     start=True, stop=False)
                    nc.tensor.matmul(pdd[:, nh * 512:(nh + 1) * 512],
                                     lhsT=zb[:, 1 * HW + j * 128: 1 * HW + (j + 1) * 128],
                                     rhs=wTp2[:, 1 * K + nh * 512: 1 * K + (nh + 1) * 512],
                                     start=False, stop=True)
                # U = fl(w2 + A) on ACT (per-partition bias), exact ref rounding
                Uj = upool.tile([128, K], f32)
                nc.scalar.activation(Uj[:], W2B[:], Act.Identity,
                                     bias=Acol[:, j:j + 1], scale=1.0)
                # td = fl(2mm - U) = -d
                td = tdpool.tile([128, K], f32)
                nc.vector.tensor_sub(td[:], pdd[:], Uj[:])
                maxv = mpool.tile([128, 8], f32)
                nc.vector.max(maxv[:], td[:])
                idx8 = ipool.tile([128, 8], u32)
                nc.vector.max_index(idx8[:], maxv[:], td[:])
                # -d_min per row -> host sums for the loss
                nc.sync.dma_start(lacc_ap[b * 8 + j, :], maxv[:, 0:1])
                nc.sync.dma_start(idx_ap[b, j * 128:(j + 1) * 128], idx8[:, 0:1].bitcast(i32))
                nc.gpsimd.indirect_dma_start(
                    out=zqg[:, j, :], out_offset=None, in_=w_ap[:, :],
                    in_offset=bass.IndirectOffsetOnAxis(ap=idx8[:, 0:1], axis=0))
            zqTb = qpool.tile([128, 2 * HW], f32)  # c-chunk cc at cols [cc*1024, +1024)
            for j in range(8):
                for cc in range(2):
                    ptt = pt.tile([128, 128], f32, tag="pt")
                    nc.tensor.transpose(ptt[:], zqg[:, j, cc * 128:(cc + 1) * 128], identity[:])
                    nc.scalar.copy(zqTb[:, cc * HW + j * 128: cc * HW + (j + 1) * 128], ptt[:])
            # straight-through output: z + (z_q - z) with reference fp32 rounding
            diff = dpool.tile([128, 2 * HW], f32)
            nc.gpsimd.tensor_sub(diff[:], zqTb[:], zb[:])
            zqo = opool.tile([128, 2 * HW], f32)
            nc.gpsimd.tensor_add(zqo[:], zb[:], diff[:])
            for cc in range(2):
                nc.sync.dma_start(zq_ap[b, cc * 128:(cc + 1) * 128, :], zqo[:, cc * HW:(cc + 1) * HW])

    _split_excess_waits(nc)
    return nc


def _get_nc():
    if "nc" not in _BUILT:
        _BUILT["nc"] = _build()
    return _BUILT["nc"]


def kernel(z, weight):
    from concourse.bass_utils import run_bass_kernel_spmd

    z = np.ascontiguousarray(np.asarray(z), dtype=np.float32)
    weight = np.ascontiguousarray(np.asarray(weight), dtype=np.float32)
    B = z.shape[0]
    assert z.shape == (B, C, 32, 32) and weight.shape == (K, C)
    zr = z.reshape(B, C, HW)

    nc = _get_nc()
    in_maps = [{"z": zr[c * B_LOC:(c + 1) * B_LOC], "w": weight} for c in range(N_CORES)]
    res = run_bass_kernel_spmd(nc, in_maps, core_ids=list(range(N_CORES)))

    zq = np.concatenate([res.results[c]["zq"] for c in range(N_CORES)], axis=0)
    zq = zq.reshape(B, C, 32, 32)
    idx = np.concatenate([res.results[c]["idx"].reshape(-1) for c in range(N_CORES)]).astype(np.int32)
    s = -sum(res.results[c]["lacc"].astype(np.float64).sum() for c in range(N_CORES))
    loss = np.float32(1.25 * s / (B * C * HW))
    return zq, loss, idx
